# revision 23
# baseline (speedup 1.0000x reference)
"""Trainium2 Bass kernel: cross-entropy with Gaussian-smoothed labels.

EXACT copy of the session-start baseline (harness-measured 122636 ns).
Restore to kernel.py if experiments end up slower.
"""
import math

import numpy as np

import concourse.bass as bass
import concourse.bacc as bacc
import concourse.tile as tile
from concourse import mybir
from concourse import bass_utils

B, T, C = 32, 2048, 722
CORES = 8
SHARD = B * T // CORES          # 8192 tokens per core
P = 128
TILES = SHARD // P              # 64
G = 4                           # token-tiles per DMA group
K = 7
START_MAX = C - K               # 715
DECAYS = [math.exp(-(2.0 ** d) / 4.0) for d in range(4)]

_ALU = mybir.AluOpType
_ACT = mybir.ActivationFunctionType

_NC = None

# cubic through f(0..3) = 1, e^-.5, e^-1, e^-2; exact at integer distances,
# negative for integer |d| >= 4 so max(.,0) kills the tail
_C3 = -0.024785177547111593
_C2 = 0.15176460742141516
_C1 = -0.5204487670682929
NDVE = 16                       # token-tiles j = 4i+2 skip the gather
DS = TILES // NDVE              # 4


def _bcast_inner(ap, n):
    """Append a step-0 broadcast dim of length n to an AP."""
    return bass.AP(tensor=ap.tensor, offset=ap.offset, ap=[*ap.ap, [0, n]])


def _build():
    nc = bacc.Bacc("TRN2", target_bir_lowering=False, debug=False,
                   enable_asserts=True, num_devices=CORES)
    pred = nc.dram_tensor("pred", [SHARD, C], mybir.dt.float32, kind="ExternalInput")
    target = nc.dram_tensor("target", [SHARD], mybir.dt.int32, kind="ExternalInput")
    out = nc.dram_tensor("partial", [P, 1], mybir.dt.float32, kind="ExternalOutput")

    pred_flat = pred.ap().rearrange("a b -> (a b)").rearrange("(n one) -> n one", one=1)
    pred_g = pred.ap().rearrange("(p j g) c -> j p g c", p=P, g=G)

    with tile.TileContext(nc) as tc:
        with (tc.tile_pool(name="pred", bufs=3) as pred_pool,
              tc.tile_pool(name="exp", bufs=4) as exp_pool,
              tc.tile_pool(name="small", bufs=1) as small):
            tgt_sb = small.tile([P, TILES], mybir.dt.int32)
            nc.sync.dma_start(out=tgt_sb, in_=target.ap().rearrange("(p j) -> p j", p=P))

            row = small.tile([P, TILES], mybir.dt.int32)
            nc.gpsimd.iota(row, pattern=[[1, TILES]], base=0, channel_multiplier=TILES)
            start_i = small.tile([P, TILES], mybir.dt.int32)
            nc.vector.tensor_scalar(out=start_i, in0=tgt_sb, scalar1=3, scalar2=0,
                                    op0=_ALU.subtract, op1=_ALU.max)
            nc.vector.tensor_scalar_min(out=start_i, in0=start_i, scalar1=START_MAX)
            offs = small.tile([P, TILES], mybir.dt.int32)
            nc.vector.tensor_scalar_mul(out=offs, in0=row, scalar1=C)
            nc.vector.tensor_add(out=offs, in0=offs, in1=start_i)

            tf = small.tile([P, TILES], mybir.dt.float32)
            nc.vector.tensor_copy(out=tf, in_=tgt_sb)
            sf = small.tile([P, TILES], mybir.dt.float32)
            nc.vector.tensor_copy(out=sf, in_=start_i)
            uf = small.tile([P, TILES], mybir.dt.float32)
            nc.vector.tensor_sub(out=uf, in0=tf, in1=sf)

            iok = small.tile([P, TILES, K], mybir.dt.float32)
            nc.gpsimd.iota(iok, pattern=[[0, TILES], [1, K]], base=0,
                           channel_multiplier=0, allow_small_or_imprecise_dtypes=True)
            diff = small.tile([P, TILES, K], mybir.dt.float32)
            nc.vector.scalar_tensor_tensor(out=diff, in0=iok, scalar=1.0,
                                           in1=_bcast_inner(uf, K),
                                           op0=_ALU.mult, op1=_ALU.subtract)
            w = small.tile([P, TILES, K], mybir.dt.float32)
            nc.vector.tensor_scalar(out=w, in0=diff, scalar1=0.0, scalar2=None,
                                    op0=_ALU.is_equal)
            tmp = small.tile([P, TILES, K], mybir.dt.float32)
            for m in (1, 2, 3):
                for s in (-m, m):
                    nc.vector.tensor_scalar(out=tmp, in0=diff, scalar1=float(s),
                                            scalar2=None, op0=_ALU.is_equal)
                    nc.vector.scalar_tensor_tensor(out=w, in0=tmp, scalar=DECAYS[m],
                                                   in1=w, op0=_ALU.mult, op1=_ALU.add)
            wsum = small.tile([P, TILES], mybir.dt.float32)
            nc.vector.reduce_sum(out=wsum, in_=w, axis=mybir.AxisListType.X)

            # 16 strided tiles (j = 4i+2) skip the serial SWDGE gather chain;
            # their window sums are computed densely from a privately
            # dual-loaded copy of their pred rows (no stream-buffer coupling):
            #   W(c) = max(cubic(|c - t|), 0)  (exact at integer distances),
            #   |c - t| on ACT (Abs with per-partition bias) filling ACT's
            #   DMA-wait gaps, the cubic + dot on DVE.
            iota722 = small.tile([P, C], mybir.dt.float32)
            nc.gpsimd.iota(iota722, pattern=[[1, C]], base=0,
                           channel_multiplier=0,
                           allow_small_or_imprecise_dtypes=True)
            neg_tf = small.tile([P, TILES], mybir.dt.float32)
            nc.vector.tensor_scalar(out=neg_tf, in0=tf, scalar1=-1.0, scalar2=None,
                                    op0=_ALU.mult)
            dpt = small.tile([P, NDVE, C], mybir.dt.float32)
            dve_src = pred.ap().rearrange("(p i e) c -> e p i c", p=P, e=DS)
            nc.scalar.dma_start(out=dpt, in_=dve_src[DS // 2])
            gsum_d = small.tile([P, TILES], mybir.dt.float32)
            nc.vector.memset(gsum_d, 0.0)
            ad0 = small.tile([P, C], mybir.dt.float32)
            ad1 = small.tile([P, C], mybir.dt.float32)
            ad = [ad0, ad1]
            t1d = small.tile([P, C], mybir.dt.float32)
            wdd = small.tile([P, C], mybir.dt.float32)
            wgd = small.tile([P, C], mybir.dt.float32)

            gath = small.tile([P, TILES, K], mybir.dt.float32)
            nc.vector.memset(gath, 0.0)
            for j in range(TILES):
                if j % DS == DS // 2:
                    continue
                nc.gpsimd.indirect_dma_start(
                    out=gath[:, j, :],
                    out_offset=None,
                    in_=pred_flat,
                    in_offset=bass.IndirectOffsetOnAxis(
                        ap=offs[:, j:j + 1], axis=0),
                )

            sums = small.tile([P, TILES], mybir.dt.float32)
            for jg in range(TILES // G):
                pt = pred_pool.tile([P, G, C], mybir.dt.float32)
                nc.sync.dma_start(out=pt, in_=pred_g[jg])
                for g in range(G):
                    j = jg * G + g
                    et = exp_pool.tile([P, C], mybir.dt.float32)
                    nc.scalar.activation(out=et, in_=pt[:, g, :], func=_ACT.Exp,
                                         accum_out=sums[:, j:j + 1])
                    if j % DS != DS // 2:
                        continue
                    i = j // DS
                    a = ad[i % 2]
                    nc.scalar.activation(out=a, in_=iota722, func=_ACT.Abs,
                                         bias=neg_tf[:, j:j + 1], scale=1.0)
                    nc.vector.tensor_scalar(out=t1d, in0=a, scalar1=_C3,
                                            scalar2=_C2, op0=_ALU.mult, op1=_ALU.add)
                    nc.vector.scalar_tensor_tensor(out=t1d, in0=t1d, scalar=0.0,
                                                   in1=a, op0=_ALU.add,
                                                   op1=_ALU.mult)
                    nc.vector.scalar_tensor_tensor(out=t1d, in0=t1d, scalar=_C1,
                                                   in1=a, op0=_ALU.add,
                                                   op1=_ALU.mult)
                    nc.vector.tensor_scalar(out=wdd, in0=t1d, scalar1=1.0,
                                            scalar2=0.0, op0=_ALU.add, op1=_ALU.max)
                    nc.vector.affine_mul_reduce(out=wgd,
                                                accum_out=gsum_d[:, j:j + 1],
                                                in0=wdd, in1=dpt[:, i, :],
                                                scale=1.0, bias=0.0)

            lse = small.tile([P, TILES], mybir.dt.float32)
            nc.scalar.activation(out=lse, in_=sums, func=_ACT.Ln)
            wg = small.tile([P, TILES, K], mybir.dt.float32)
            gsum = small.tile([P, TILES], mybir.dt.float32)
            nc.vector.tensor_mul(out=wg, in0=w, in1=gath)
            nc.vector.reduce_sum(out=gsum, in_=wg, axis=mybir.AxisListType.X)
            nc.vector.tensor_add(out=gsum, in0=gsum, in1=gsum_d)
            loss = small.tile([P, TILES], mybir.dt.float32)
            nc.vector.tensor_mul(out=loss, in0=wsum, in1=lse)
            nc.vector.tensor_sub(out=loss, in0=loss, in1=gsum)
            part = small.tile([P, 1], mybir.dt.float32)
            nc.vector.reduce_sum(out=part, in_=loss, axis=mybir.AxisListType.X)
            nc.sync.dma_start(out=out.ap(), in_=part)
    nc.compile()
    return nc


def _get_nc():
    global _NC
    if _NC is None:
        _NC = _build()
    return _NC


def _shard_inputs(pred, target):
    bpc = B // CORES
    in_maps = []
    for c in range(CORES):
        in_maps.append({
            "pred": np.ascontiguousarray(
                pred[c * bpc:(c + 1) * bpc].reshape(SHARD, C), dtype=np.float32),
            "target": np.ascontiguousarray(
                target[c * bpc:(c + 1) * bpc].reshape(SHARD), dtype=np.int32),
        })
    return in_maps


def _run(pred, target, **kwargs):
    nc = _get_nc()
    return bass_utils.run_bass_kernel_spmd(
        nc, _shard_inputs(pred, target), core_ids=list(range(CORES)), **kwargs)


def kernel(pred, target):
    res = _run(pred, target)
    total = sum(float(r["partial"].astype(np.float64).sum()) for r in res.results)
    return np.asarray(total / (B * T), dtype=np.float32)


# revision 26
# speedup vs baseline: 1.3084x; 1.3084x over previous
"""Trainium2 Bass kernel: cross-entropy with Gaussian-smoothed labels.

EXACT copy of the session-start baseline (harness-measured 122636 ns).
Restore to kernel.py if experiments end up slower.
"""
import math

import numpy as np

import concourse.bass as bass
import concourse.bacc as bacc
import concourse.tile as tile
from concourse import mybir
from concourse import bass_utils

B, T, C = 32, 2048, 722
CORES = 8
SHARD = B * T // CORES          # 8192 tokens per core
P = 128
TILES = SHARD // P              # 64
G = 4                           # token-tiles per DMA group
K = 7
START_MAX = C - K               # 715
DECAYS = [math.exp(-(2.0 ** d) / 4.0) for d in range(4)]

_ALU = mybir.AluOpType
_ACT = mybir.ActivationFunctionType

_NC = None


def _bcast_inner(ap, n):
    """Append a step-0 broadcast dim of length n to an AP."""
    return bass.AP(tensor=ap.tensor, offset=ap.offset, ap=[*ap.ap, [0, n]])


def _build():
    nc = bacc.Bacc("TRN2", target_bir_lowering=False, debug=False,
                   enable_asserts=True, num_devices=CORES)
    pred = nc.dram_tensor("pred", [SHARD, C], mybir.dt.float32, kind="ExternalInput")
    target = nc.dram_tensor("target", [SHARD], mybir.dt.int32, kind="ExternalInput")
    out = nc.dram_tensor("partial", [P, 1], mybir.dt.float32, kind="ExternalOutput")

    pred_flat = pred.ap().rearrange("a b -> (a b)").rearrange("(n one) -> n one", one=1)
    pred_g = pred.ap().rearrange("(p j g) c -> j p g c", p=P, g=G)

    with tile.TileContext(nc) as tc:
        with (tc.tile_pool(name="pred", bufs=3) as pred_pool,
              tc.tile_pool(name="exp", bufs=4) as exp_pool,
              tc.tile_pool(name="small", bufs=1) as small):
            tgt_sb = small.tile([P, TILES], mybir.dt.int32)
            nc.sync.dma_start(out=tgt_sb, in_=target.ap().rearrange("(p j) -> p j", p=P))

            row = small.tile([P, TILES], mybir.dt.int32)
            nc.gpsimd.iota(row, pattern=[[1, TILES]], base=0, channel_multiplier=TILES)
            start_i = small.tile([P, TILES], mybir.dt.int32)
            nc.vector.tensor_scalar(out=start_i, in0=tgt_sb, scalar1=3, scalar2=0,
                                    op0=_ALU.subtract, op1=_ALU.max)
            nc.vector.tensor_scalar_min(out=start_i, in0=start_i, scalar1=START_MAX)
            offs = small.tile([P, TILES], mybir.dt.int32)
            nc.vector.tensor_scalar_mul(out=offs, in0=row, scalar1=C)
            nc.vector.tensor_add(out=offs, in0=offs, in1=start_i)

            tf = small.tile([P, TILES], mybir.dt.float32)
            nc.vector.tensor_copy(out=tf, in_=tgt_sb)
            sf = small.tile([P, TILES], mybir.dt.float32)
            nc.vector.tensor_copy(out=sf, in_=start_i)
            uf = small.tile([P, TILES], mybir.dt.float32)
            nc.vector.tensor_sub(out=uf, in0=tf, in1=sf)

            iok = small.tile([P, TILES, K], mybir.dt.float32)
            nc.gpsimd.iota(iok, pattern=[[0, TILES], [1, K]], base=0,
                           channel_multiplier=0, allow_small_or_imprecise_dtypes=True)
            diff = small.tile([P, TILES, K], mybir.dt.float32)
            nc.vector.scalar_tensor_tensor(out=diff, in0=iok, scalar=1.0,
                                           in1=_bcast_inner(uf, K),
                                           op0=_ALU.mult, op1=_ALU.subtract)
            w = small.tile([P, TILES, K], mybir.dt.float32)
            nc.vector.tensor_scalar(out=w, in0=diff, scalar1=0.0, scalar2=None,
                                    op0=_ALU.is_equal)
            tmp = small.tile([P, TILES, K], mybir.dt.float32)
            for m in (1, 2, 3):
                for s in (-m, m):
                    nc.vector.tensor_scalar(out=tmp, in0=diff, scalar1=float(s),
                                            scalar2=None, op0=_ALU.is_equal)
                    nc.vector.scalar_tensor_tensor(out=w, in0=tmp, scalar=DECAYS[m],
                                                   in1=w, op0=_ALU.mult, op1=_ALU.add)
            wsum = small.tile([P, TILES], mybir.dt.float32)
            nc.vector.reduce_sum(out=wsum, in_=w, axis=mybir.AxisListType.X)

            # 8 strided tiles (j = 8i+4) skip the serial SWDGE gather chain
            # (the kernel's critical path); their window sums come from a
            # dense dot with W(c) = max(cubic(|c-t|), 0) -- exact at integer
            # distances -- against privately dual-loaded pred rows, all on
            # DVE with no stream or ACT coupling.
            iota722 = small.tile([P, C], mybir.dt.float32)
            nc.gpsimd.iota(iota722, pattern=[[1, C]], base=0,
                           channel_multiplier=0,
                           allow_small_or_imprecise_dtypes=True)
            dpt = small.tile([P, 8, C], mybir.dt.float32)
            dve_src = pred.ap().rearrange("(p i e) c -> e p i c", p=P, e=8)
            nc.scalar.dma_start(out=dpt, in_=dve_src[4])
            gsum_d = small.tile([P, TILES], mybir.dt.float32)
            nc.vector.memset(gsum_d, 0.0)
            diffd = small.tile([P, C], mybir.dt.float32)
            add = small.tile([P, C], mybir.dt.float32)
            t1d = small.tile([P, C], mybir.dt.float32)
            wdd = small.tile([P, C], mybir.dt.float32)
            wgd = small.tile([P, C], mybir.dt.float32)

            gath = small.tile([P, TILES, K], mybir.dt.float32)
            for i in range(8):
                nc.vector.memset(gath[:, 8 * i + 4, :], 0.0)
            for i in range(8):
                j = 8 * i + 4
                nc.vector.tensor_scalar(out=diffd, in0=iota722,
                                        scalar1=tf[:, j:j + 1], scalar2=None,
                                        op0=_ALU.subtract)
                nc.vector.scalar_tensor_tensor(out=add, in0=diffd, scalar=-1.0,
                                               in1=diffd, op0=_ALU.mult,
                                               op1=_ALU.max)
                nc.vector.tensor_scalar(out=t1d, in0=add, scalar1=-0.024785177547111593,
                                        scalar2=0.15176460742141516,
                                        op0=_ALU.mult, op1=_ALU.add)
                nc.vector.scalar_tensor_tensor(out=t1d, in0=t1d, scalar=0.0,
                                               in1=add, op0=_ALU.add, op1=_ALU.mult)
                nc.vector.scalar_tensor_tensor(out=t1d, in0=t1d,
                                               scalar=-0.5204487670682929,
                                               in1=add, op0=_ALU.add, op1=_ALU.mult)
                nc.vector.tensor_scalar(out=wdd, in0=t1d, scalar1=1.0,
                                        scalar2=0.0, op0=_ALU.add, op1=_ALU.max)
                nc.vector.affine_mul_reduce(out=wgd, accum_out=gsum_d[:, j:j + 1],
                                            in0=wdd, in1=dpt[:, i, :],
                                            scale=1.0, bias=0.0)

            for j in range(TILES):
                if j % 8 == 4:
                    continue
                nc.gpsimd.indirect_dma_start(
                    out=gath[:, j, :],
                    out_offset=None,
                    in_=pred_flat,
                    in_offset=bass.IndirectOffsetOnAxis(
                        ap=offs[:, j:j + 1], axis=0),
                )

            sums = small.tile([P, TILES], mybir.dt.float32)
            for jg in range(TILES // G):
                pt = pred_pool.tile([P, G, C], mybir.dt.float32)
                nc.sync.dma_start(out=pt, in_=pred_g[jg])
                for g in range(G):
                    j = jg * G + g
                    et = exp_pool.tile([P, C], mybir.dt.float32)
                    nc.scalar.activation(out=et, in_=pt[:, g, :], func=_ACT.Exp,
                                         accum_out=sums[:, j:j + 1])

            lse = small.tile([P, TILES], mybir.dt.float32)
            nc.scalar.activation(out=lse, in_=sums, func=_ACT.Ln)
            wg = small.tile([P, TILES, K], mybir.dt.float32)
            gsum = small.tile([P, TILES], mybir.dt.float32)
            nc.vector.tensor_mul(out=wg, in0=w, in1=gath)
            nc.vector.reduce_sum(out=gsum, in_=wg, axis=mybir.AxisListType.X)
            nc.vector.tensor_add(out=gsum, in0=gsum, in1=gsum_d)
            loss = small.tile([P, TILES], mybir.dt.float32)
            nc.vector.tensor_mul(out=loss, in0=wsum, in1=lse)
            nc.vector.tensor_sub(out=loss, in0=loss, in1=gsum)
            part = small.tile([P, 1], mybir.dt.float32)
            nc.vector.reduce_sum(out=part, in_=loss, axis=mybir.AxisListType.X)
            nc.sync.dma_start(out=out.ap(), in_=part)
    nc.compile()
    return nc


def _get_nc():
    global _NC
    if _NC is None:
        _NC = _build()
    return _NC


def _shard_inputs(pred, target):
    bpc = B // CORES
    in_maps = []
    for c in range(CORES):
        in_maps.append({
            "pred": np.ascontiguousarray(
                pred[c * bpc:(c + 1) * bpc].reshape(SHARD, C), dtype=np.float32),
            "target": np.ascontiguousarray(
                target[c * bpc:(c + 1) * bpc].reshape(SHARD), dtype=np.int32),
        })
    return in_maps


def _run(pred, target, **kwargs):
    nc = _get_nc()
    return bass_utils.run_bass_kernel_spmd(
        nc, _shard_inputs(pred, target), core_ids=list(range(CORES)), **kwargs)


def kernel(pred, target):
    res = _run(pred, target)
    total = sum(float(r["partial"].astype(np.float64).sum()) for r in res.results)
    return np.asarray(total / (B * T), dtype=np.float32)
